# revision 10
# baseline (speedup 1.0000x reference)
"""Trainium2 Bass kernel for nn_CNN_symmetry (dense_cnn).

Strategy v2:
  * Pure data parallelism: B=32768 sharded across 8 NeuronCores (4096 each).
  * Per core: 4 "dblocks" of 1024 batch cols. Main 128 pixels as [128, 1024]
    tiles; the 16 tail pixels of all 4 dblocks PACKED into one [128, 1024]
    tile at 32-stride partition slots (strip j = dblock j), so all tail
    elementwise work runs once per core instead of once per dblock.
  * Tail conv outputs land partition-packed in PSUM via tile_position
    col-strips (mt: (0,32j), tt: (32j,32j)); tail conv inputs feed via
    row-strips (tm: (32j,0)). Col-strip matmuls run concurrently on the PE.
  * Masked selects via copy_predicated reading PSUM directly (int16 views
    of bf16 masks). MLP lrelu/bias fused into ScalarE activations.

Algebra (same math as reference, restructured):
    e=[x==0], m_c=[x==c], ie=1-e; C_sum=C_each+C_ne; T=C_ne@1
    t0 = T - C_ne@e ; E0 = C_emp@e ; ew = E0 - t0 ; ebar = ie*E0
    necn_c = C_ne@m_c - t0
    s0 = sum_c m_c*(C_sum@m_c) + ie*ew ; allv = sigmoid(s0)
    2x: asel = sum_c m_c*(C_ne2@(necn_c*allv)); s += ebar - asel
        allv = tanh(s/2)
    out = lrelu(lrelu(allv@W1')@W2'+b2)@W3'+b3
"""

import os
import sys
from contextlib import ExitStack

import numpy as np

sys.path.insert(0, "/opt/trn_rl_repo")
os.environ.setdefault("MYCRO_LOCAL_CACHE", "1")

import ml_dtypes  # noqa: E402

import concourse.bass as bass  # noqa: E402
import concourse.bacc as bacc  # noqa: E402
import concourse.tile as tile  # noqa: E402
from concourse import mybir  # noqa: E402

V, H, B = 14, 12, 32768
NK, CT = 5, 4
HALF, FULL = 6, 13
NPIX, NSQ = 144, 100
NCORES = 8
BC = B // NCORES          # 4096 per core
DB = 1024                 # dblock width
NDB = BC // DB            # 4
PM, PT = 128, 16          # main/tail pixel split (i-major order p = i*12 + j)

BF16 = mybir.dt.bfloat16
F32 = mybir.dt.float32
I16 = mybir.dt.int16
AF = mybir.ActivationFunctionType
ALU = mybir.AluOpType


# ---------------------------------------------------------------- host prep

def _build_K(W):
    Wa = np.abs(np.asarray(W, np.float64))
    K = np.zeros((FULL, FULL))
    K[:, HALF:HALF + CT] = Wa
    K[:, HALF - CT + 1:HALF + 1] = Wa[:, ::-1]
    return K


def _band(K):
    C = np.zeros((NPIX, NPIX))
    for i in range(12):
        for j in range(12):
            for i2 in range(12):
                for j2 in range(12):
                    di, dj = i2 - i + HALF, j2 - j + HALF
                    if 0 <= di < FULL and 0 <= dj < FULL:
                        C[i * 12 + j, i2 * 12 + j2] = K[di, dj]
    return C


def _strip4(block16):
    """[16, W] -> [128, W] with copies at partition offsets 0/32/64/96."""
    W = block16.shape[1]
    out = np.zeros((PM, W), block16.dtype)
    for j in range(4):
        out[32 * j:32 * j + PT] = block16
    return out


def build_consts(w_each, w_not_each, w_not_each_2nd, w_empty, W1, W2, b2, W3, b3):
    C_each = _band(_build_K(w_each))
    C_ne = _band(_build_K(w_not_each))
    C_ne2 = _band(_build_K(w_not_each_2nd))
    C_emp = _band(_build_K(w_empty))
    C_sum = C_each + C_ne
    T = C_ne @ np.ones(NPIX)

    bf = lambda a: np.ascontiguousarray(np.asarray(a), dtype=ml_dtypes.bfloat16)
    f32 = lambda a: np.ascontiguousarray(np.asarray(a), dtype=np.float32)

    consts = {}
    for name, C in (("csum", C_sum), ("cne", C_ne), ("cemp", C_emp), ("cne2", C_ne2)):
        L = C.T  # lhsT [in, out]
        consts[f"{name}_mm"] = bf(L[:PM, :PM])
        consts[f"{name}_tm4"] = bf(_strip4(L[PM:, :PM]))       # [128,128]
        consts[f"{name}_mt"] = bf(L[:PM, PM:])                 # [128,16]
        consts[f"{name}_tt4"] = bf(_strip4(L[PM:, PM:]))       # [128,16]
    consts["t_m"] = f32(T[:PM].reshape(PM, 1))
    consts["t_t4"] = f32(_strip4(T[PM:].reshape(PT, 1).astype(np.float32)))
    W1T = np.asarray(W1, np.float64).T        # [144, 100]
    consts["w1_m"] = bf(W1T[:PM])
    consts["w1_t4"] = bf(_strip4(W1T[PM:].astype(np.float64)))  # [128, 100]
    consts["w2"] = bf(np.asarray(W2, np.float64).T)   # [100, 100]
    consts["w3"] = bf(np.asarray(W3, np.float64).T)   # [100, 1]
    consts["b2"] = f32(np.asarray(b2).reshape(NSQ, 1))
    consts["b3"] = f32(np.asarray(b3).reshape(1, 1))
    return consts


CONST_SPECS = (
    [(f"{n}_{bk}", shp, BF16)
     for n in ("csum", "cne", "cemp", "cne2")
     for bk, shp in (("mm", [PM, PM]), ("tm4", [PM, PM]),
                     ("mt", [PM, PT]), ("tt4", [PM, PT]))]
    + [("t_m", [PM, 1], F32), ("t_t4", [PM, 1], F32),
       ("w1_m", [PM, NSQ], BF16), ("w1_t4", [PM, NSQ], BF16),
       ("w2", [NSQ, NSQ], BF16), ("w3", [NSQ, 1], BF16),
       ("b2", [NSQ, 1], F32), ("b3", [1, 1], F32)]
)


# ---------------------------------------------------------------- device kernel

def emit_kernel(nc):
    xm_d = nc.dram_tensor("xm", [PM, BC], BF16, kind="ExternalInput")
    xt_d = nc.dram_tensor("xt4", [PM, DB], BF16, kind="ExternalInput")
    out_d = nc.dram_tensor("out", [1, BC], F32, kind="ExternalOutput")
    const_d = {n: nc.dram_tensor(n, shp, dt, kind="ExternalInput")
               for n, shp, dt in CONST_SPECS}

    with tile.TileContext(nc) as tc, ExitStack() as ctx:
        cpool = ctx.enter_context(tc.tile_pool(name="consts", bufs=1))
        xpool = ctx.enter_context(tc.tile_pool(name="x", bufs=1))
        qpool = ctx.enter_context(tc.tile_pool(name="perq", bufs=1))
        shpool = ctx.enter_context(tc.tile_pool(name="shared", bufs=2))
        npool = ctx.enter_context(tc.tile_pool(name="nscr", bufs=3))
        gpool = ctx.enter_context(tc.tile_pool(name="g", bufs=1))
        opool = ctx.enter_context(tc.tile_pool(name="outs", bufs=1))
        ppool = ctx.enter_context(tc.tile_pool(name="ps", bufs=1, space="PSUM"))
        tpool = ctx.enter_context(tc.tile_pool(name="pst", bufs=1, space="PSUM"))

        # ACT table warmup with no data deps
        warm = cpool.tile([1, 1], F32, tag="warm", name="warm")
        nc.vector.memset(warm[:], 0.0)
        nc.scalar.activation(warm[:], warm[:], AF.Copy)
        nc.scalar.activation(warm[:], warm[:], AF.Sigmoid)
        nc.scalar.activation(warm[:], warm[:], AF.Tanh)

        C = {}
        for n, shp, dt in CONST_SPECS:
            t = cpool.tile(shp, dt, tag=n, name=n)
            nc.gpsimd.dma_start(t[:], const_d[n][:])
            C[n] = t

        xm = xpool.tile([PM, BC], BF16, tag="xm", name="xm_t")
        xt = xpool.tile([PM, DB], BF16, tag="xt", name="xt_t")
        nc.gpsimd.dma_start(xm[:], xm_d[:])
        nc.gpsimd.dma_start(xt[:], xt_d[:])

        NQ = NDB + 1   # 4 main dblocks + 1 tail set (index NDB)

        def xq(q):
            # x source for set q: main slice or tail pack
            return xm[:, q * DB:(q + 1) * DB] if q < NDB else xt[:]

        # ---- persistent per-set tiles -------------------------------------
        mstk = [qpool.tile([PM, NK * DB], BF16, tag=f"mstk{q}", name=f"mstk{q}")
                for q in range(NQ)]          # masks m_1..m_5 stacked on free dim
        necn = [qpool.tile([PM, NK * DB], BF16, tag=f"necn{q}", name=f"necn{q}")
                for q in range(NQ)]
        s_t = [qpool.tile([PM, DB], BF16, tag=f"s{q}", name=f"s{q}")
               for q in range(NQ)]
        allv = [qpool.tile([PM, DB], BF16, tag=f"allv{q}", name=f"allv{q}")
                for q in range(NQ)]
        ebar = [qpool.tile([PM, DB], BF16, tag=f"ebar{q}", name=f"ebar{q}")
                for q in range(NQ)]
        sel = [qpool.tile([PM, DB], BF16, tag=f"sel{q}", name=f"sel{q}")
               for q in range(NQ)]
        t0_t = [qpool.tile([PM, DB], BF16, tag=f"t0{q}", name=f"t0{q}")
                for q in range(NQ)]

        def msl(q, c):
            return mstk[q][:, (c - 1) * DB:c * DB]

        def mint(q, c):
            return msl(q, c).bitcast(I16)

        # ---- conv stage helper --------------------------------------------
        def conv_stage(mat, rhs_main, rhs_tail, evac_main, evac_tail):
            """rhs_main(j)->AP [128, DB]; rhs_tail->AP [128, DB] pack (strips).
            evac_main(j, ps), evac_tail(ps)."""
            for j in range(NDB):
                ps = ppool.tile([PM, DB], F32, tag=f"pm{j % 3}", name=f"pm{j % 3}")
                rm = rhs_main(j)
                for nn in (0, 512):
                    nc.tensor.matmul(ps[:, nn:nn + 512], C[f"{mat}_mm"][:],
                                     rm[:, nn:nn + 512], start=True, stop=False)
                for nn in (0, 512):
                    nc.tensor.matmul(ps[:, nn:nn + 512],
                                     C[f"{mat}_tm4"][32 * j:32 * j + PT, :],
                                     rhs_tail[32 * j:32 * j + PT, nn:nn + 512],
                                     start=False, stop=True,
                                     tile_position=(32 * j, 0),
                                     skip_group_check=True)
                evac_main(j, ps)
            pst = tpool.tile([PM, DB], F32, tag="pt", name="pt")
            for nn in (0, 512):
                for j in range(NDB):
                    rm = rhs_main(j)
                    nc.tensor.matmul(pst[32 * j:32 * j + PT, nn:nn + 512],
                                     C[f"{mat}_mt"][:],
                                     rm[:, nn:nn + 512], start=True, stop=False,
                                     tile_position=(0, 32 * j),
                                     skip_group_check=True)
                for j in range(NDB):
                    nc.tensor.matmul(pst[32 * j:32 * j + PT, nn:nn + 512],
                                     C[f"{mat}_tt4"][32 * j:32 * j + PT, :],
                                     rhs_tail[32 * j:32 * j + PT, nn:nn + 512],
                                     start=False, stop=True,
                                     tile_position=(32 * j, 32 * j),
                                     skip_group_check=True)
            evac_tail(pst)

        # ---- phase A: masks ----------------------------------------------
        e_t = [qpool.tile([PM, DB], BF16, tag=f"e{q}", name=f"e{q}")
               for q in range(NQ)]
        for q in range(NQ):
            x = xq(q)
            eng = nc.gpsimd if q in (1, 3) else nc.vector
            eng.tensor_scalar(e_t[q][:], x, 0, None, ALU.is_equal)
            for c in range(1, NK + 1):
                eng.tensor_scalar(msl(q, c), x, c, None, ALU.is_equal)

        # ---- phase B: e-convs --------------------------------------------
        def ev_t0_m(j, ps):
            nc.scalar.activation(t0_t[j][:], ps[:], AF.Identity,
                                 bias=C["t_m"][:], scale=-1.0)

        def ev_t0_t(ps):
            nc.scalar.activation(t0_t[NDB][:], ps[:], AF.Identity,
                                 bias=C["t_t4"][:], scale=-1.0)

        conv_stage("cne", lambda j: e_t[j][:], e_t[NDB][:], ev_t0_m, ev_t0_t)

        def ev_e0(q, ps):
            e0 = shpool.tile([PM, DB], BF16, tag="e0", name=f"e0{q}")
            ie = shpool.tile([PM, DB], BF16, tag="ie", name=f"ie{q}")
            nc.scalar.activation(e0[:], ps[:], AF.Copy)
            nc.vector.tensor_scalar(ie[:], xq(q), 0, None, ALU.not_equal)
            # ebar = ie*E0 ; ew = E0-t0 (in place) ; s = ie*ew
            nc.vector.tensor_tensor(ebar[q][:], ie[:], e0[:], ALU.mult)
            nc.vector.tensor_tensor(e0[:], e0[:], t0_t[q][:], ALU.subtract)
            nc.vector.tensor_tensor(s_t[q][:], ie[:], e0[:], ALU.mult)

        conv_stage("cemp", lambda j: e_t[j][:], e_t[NDB][:],
                   lambda j, ps: ev_e0(j, ps), lambda ps: ev_e0(NDB, ps))

        # ---- phase C: per-color convs ------------------------------------
        for c in range(1, NK + 1):
            def ev_n(q, ps, c=c):
                nscr = npool.tile([PM, DB], BF16, tag="n", name=f"n{q}")
                nc.scalar.activation(nscr[:], ps[:], AF.Copy)
                nc.vector.tensor_tensor(necn[q][:, (c - 1) * DB:c * DB],
                                        nscr[:], t0_t[q][:], ALU.subtract)

            conv_stage("cne", lambda j, c=c: msl(j, c), msl(NDB, c),
                       lambda j, ps: ev_n(j, ps), lambda ps: ev_n(NDB, ps))

            def ev_u(q, ps, c=c):
                if c == 1:
                    nc.vector.tensor_tensor(sel[q][:], msl(q, c), ps[:], ALU.mult)
                else:
                    nc.vector.copy_predicated(sel[q][:], mint(q, c), ps[:])

            conv_stage("csum", lambda j, c=c: msl(j, c), msl(NDB, c),
                       lambda j, ps: ev_u(j, ps), lambda ps: ev_u(NDB, ps))

        # ---- s0 / sigmoid -------------------------------------------------
        for q in range(NQ):
            nc.vector.tensor_tensor(s_t[q][:], s_t[q][:], sel[q][:], ALU.add)
            nc.scalar.activation(allv[q][:], s_t[q][:], AF.Sigmoid)

        # ---- depth loop x2 ------------------------------------------------
        for it in range(2):
            for c in range(1, NK + 1):
                gt = [gpool.tile([PM, DB], BF16, tag=f"g{q}", name=f"g{q}")
                      for q in range(NQ)]
                for q in range(NQ):
                    nc.vector.tensor_tensor(gt[q][:],
                                            necn[q][:, (c - 1) * DB:c * DB],
                                            allv[q][:], ALU.mult)

                def ev_a(q, ps, c=c):
                    if c == 1:
                        nc.vector.tensor_tensor(sel[q][:], msl(q, c), ps[:],
                                                ALU.mult)
                    else:
                        nc.vector.copy_predicated(sel[q][:], mint(q, c), ps[:])

                conv_stage("cne2", lambda j: gt[j][:], gt[NDB][:],
                           lambda j, ps: ev_a(j, ps), lambda ps: ev_a(NDB, ps))
            for q in range(NQ):
                nc.vector.tensor_tensor(s_t[q][:], s_t[q][:], ebar[q][:], ALU.add)
                nc.vector.tensor_tensor(s_t[q][:], s_t[q][:], sel[q][:], ALU.subtract)
                nc.scalar.activation(allv[q][:], s_t[q][:], AF.Tanh, scale=0.5)

        # ---- MLP ----------------------------------------------------------
        for j in range(NDB):
            ps1 = ppool.tile([PM, DB], F32, tag=f"pm{j % 3}", name=f"mlp1_{j}")
            for nn in (0, 512):
                nc.tensor.matmul(ps1[:NSQ, nn:nn + 512], C["w1_m"][:],
                                 allv[j][:, nn:nn + 512], start=True, stop=False)
                nc.tensor.matmul(ps1[:NSQ, nn:nn + 512],
                                 C["w1_t4"][32 * j:32 * j + PT, :],
                                 allv[NDB][32 * j:32 * j + PT, nn:nn + 512],
                                 start=False, stop=True,
                                 tile_position=(32 * j, 0),
                                 skip_group_check=True)
            h1 = opool.tile([NSQ, DB], BF16, tag="h1", name="h1")
            nc.scalar.activation(h1[:], ps1[:NSQ], AF.Copy)
            nc.vector.scalar_tensor_tensor(h1[:], ps1[:NSQ], 0.2, h1[:],
                                           ALU.mult, ALU.max)
            ps2 = tpool.tile([PM, DB], F32, tag="pt", name=f"mlp2_{j}")
            for nn in (0, 512):
                nc.tensor.matmul(ps2[:NSQ, nn:nn + 512], C["w2"][:],
                                 h1[:, nn:nn + 512], start=True, stop=True)
            h2 = opool.tile([NSQ, DB], BF16, tag="h2", name="h2")
            nc.scalar.activation(h2[:], ps2[:NSQ], AF.Identity, bias=C["b2"][:])
            nc.vector.scalar_tensor_tensor(h2[:], h2[:], 0.2, h2[:],
                                           ALU.mult, ALU.max)
            ps3 = ppool.tile([PM, DB], F32, tag=f"pm{(j + 1) % 3}", name=f"mlp3_{j}")
            for nn in (0, 512):
                nc.tensor.matmul(ps3[:1, nn:nn + 512], C["w3"][:],
                                 h2[:, nn:nn + 512], start=True, stop=True)
            yout = opool.tile([1, DB], F32, tag="yout", name="yout")
            nc.scalar.activation(yout[:], ps3[:1], AF.Identity, bias=C["b3"][:])
            nc.gpsimd.dma_start(out_d[:, j * DB:(j + 1) * DB], yout[:])

    return nc


# ---------------------------------------------------------------- entry point

def _prep_inputs(dots):
    """dots (14,12,B) int32 -> per-core xm [128, BC] bf16, xt4 [128, DB] bf16."""
    x = np.asarray(dots)[:12].reshape(NPIX, B).astype(ml_dtypes.bfloat16)
    xms, xts = [], []
    for k in range(NCORES):
        sl = x[:, k * BC:(k + 1) * BC]
        xms.append(np.ascontiguousarray(sl[:PM]))
        tail = sl[PM:]                       # [16, 4096]
        pack = np.zeros((PM, DB), ml_dtypes.bfloat16)
        for j in range(NDB):
            pack[32 * j:32 * j + PT] = tail[:, j * DB:(j + 1) * DB]
        xts.append(pack)
    return xms, xts


def kernel(dots, w_each, w_not_each, w_not_each_2nd, w_empty, W1, W2, b2, W3, b3):
    from concourse.bass_utils import run_bass_kernel_spmd

    consts = build_consts(w_each, w_not_each, w_not_each_2nd, w_empty,
                          W1, W2, b2, W3, b3)
    xms, xts = _prep_inputs(dots)

    nc = bacc.Bacc()
    emit_kernel(nc)
    nc.compile()

    in_maps = [dict(consts, xm=xms[k], xt4=xts[k]) for k in range(NCORES)]
    res = run_bass_kernel_spmd(nc, in_maps, list(range(NCORES)))
    out = np.concatenate([np.asarray(r["out"]).reshape(BC) for r in res.results])
    return out.reshape(B, 1).astype(np.float32)


if __name__ == "__main__":
    rng = np.random.default_rng(0)
    ins = {
        "dots": rng.integers(0, 6, size=(V, H, B)).astype(np.int32),
        "w_each": rng.standard_normal((FULL, CT), dtype=np.float32) * 0.1,
        "w_not_each": rng.standard_normal((FULL, CT), dtype=np.float32) * 0.1,
        "w_not_each_2nd": rng.standard_normal((FULL, CT), dtype=np.float32) * 0.1,
        "w_empty": rng.standard_normal((FULL, CT), dtype=np.float32) * 0.1,
        "W1": rng.standard_normal((NSQ, NPIX), dtype=np.float32) * 0.2,
        "W2": rng.standard_normal((NSQ, NSQ), dtype=np.float32) * 0.2,
        "b2": rng.standard_normal(NSQ, dtype=np.float32) * 0.1,
        "W3": rng.standard_normal((1, NSQ), dtype=np.float32) * 0.2,
        "b3": rng.standard_normal(1, dtype=np.float32) * 0.1,
    }
    y = kernel(**ins)
    print("kernel out", y.shape, y[:4, 0])


# revision 11
# speedup vs baseline: 1.5554x; 1.5554x over previous
"""Trainium2 Bass kernel for nn_CNN_symmetry (dense_cnn).

Strategy v2:
  * Pure data parallelism: B=32768 sharded across 8 NeuronCores (4096 each).
  * Per core: 4 "dblocks" of 1024 batch cols. Main 128 pixels as [128, 1024]
    tiles; the 16 tail pixels of all 4 dblocks PACKED into one [128, 1024]
    tile at 32-stride partition slots (strip j = dblock j), so all tail
    elementwise work runs once per core instead of once per dblock.
  * Tail conv outputs land partition-packed in PSUM via tile_position
    col-strips (mt: (0,32j), tt: (32j,32j)); tail conv inputs feed via
    row-strips (tm: (32j,0)). Col-strip matmuls run concurrently on the PE.
  * Masked selects via copy_predicated reading PSUM directly (int16 views
    of bf16 masks). MLP lrelu/bias fused into ScalarE activations.

Algebra (same math as reference, restructured):
    e=[x==0], m_c=[x==c], ie=1-e; C_sum=C_each+C_ne; T=C_ne@1
    t0 = T - C_ne@e ; E0 = C_emp@e ; ew = E0 - t0 ; ebar = ie*E0
    necn_c = C_ne@m_c - t0
    s0 = sum_c m_c*(C_sum@m_c) + ie*ew ; allv = sigmoid(s0)
    2x: asel = sum_c m_c*(C_ne2@(necn_c*allv)); s += ebar - asel
        allv = tanh(s/2)
    out = lrelu(lrelu(allv@W1')@W2'+b2)@W3'+b3
"""

import os
import sys
from contextlib import ExitStack

import numpy as np

sys.path.insert(0, "/opt/trn_rl_repo")
os.environ.setdefault("MYCRO_LOCAL_CACHE", "1")

import ml_dtypes  # noqa: E402

import concourse.bass as bass  # noqa: E402
import concourse.bacc as bacc  # noqa: E402
import concourse.tile as tile  # noqa: E402
from concourse import mybir  # noqa: E402

V, H, B = 14, 12, 32768
NK, CT = 5, 4
HALF, FULL = 6, 13
NPIX, NSQ = 144, 100
NCORES = 8
BC = B // NCORES          # 4096 per core
DB = 1024                 # dblock width
NDB = BC // DB            # 4
PM, PT = 128, 16          # main/tail pixel split (i-major order p = i*12 + j)

BF16 = mybir.dt.bfloat16
F32 = mybir.dt.float32
I16 = mybir.dt.int16
AF = mybir.ActivationFunctionType
ALU = mybir.AluOpType


# ---------------------------------------------------------------- host prep

def _build_K(W):
    Wa = np.abs(np.asarray(W, np.float64))
    K = np.zeros((FULL, FULL))
    K[:, HALF:HALF + CT] = Wa
    K[:, HALF - CT + 1:HALF + 1] = Wa[:, ::-1]
    return K


def _band(K):
    C = np.zeros((NPIX, NPIX))
    for i in range(12):
        for j in range(12):
            for i2 in range(12):
                for j2 in range(12):
                    di, dj = i2 - i + HALF, j2 - j + HALF
                    if 0 <= di < FULL and 0 <= dj < FULL:
                        C[i * 12 + j, i2 * 12 + j2] = K[di, dj]
    return C


def _strip4(block16):
    """[16, W] -> [128, W] with copies at partition offsets 0/32/64/96."""
    W = block16.shape[1]
    out = np.zeros((PM, W), block16.dtype)
    for j in range(4):
        out[32 * j:32 * j + PT] = block16
    return out


def build_consts(w_each, w_not_each, w_not_each_2nd, w_empty, W1, W2, b2, W3, b3):
    C_each = _band(_build_K(w_each))
    C_ne = _band(_build_K(w_not_each))
    C_ne2 = _band(_build_K(w_not_each_2nd))
    C_emp = _band(_build_K(w_empty))
    C_sum = C_each + C_ne
    T = C_ne @ np.ones(NPIX)

    bf = lambda a: np.ascontiguousarray(np.asarray(a), dtype=ml_dtypes.bfloat16)
    f32 = lambda a: np.ascontiguousarray(np.asarray(a), dtype=np.float32)

    consts = {}
    for name, C in (("csum", C_sum), ("cne", C_ne), ("cemp", C_emp), ("cne2", C_ne2)):
        L = C.T  # lhsT [in, out]
        consts[f"{name}_mm"] = bf(L[:PM, :PM])
        consts[f"{name}_tm4"] = bf(_strip4(L[PM:, :PM]))       # [128,128]
        consts[f"{name}_mt"] = bf(L[:PM, PM:])                 # [128,16]
        consts[f"{name}_tt4"] = bf(_strip4(L[PM:, PM:]))       # [128,16]
    consts["t_m"] = f32(T[:PM].reshape(PM, 1))
    consts["t_t4"] = f32(_strip4(T[PM:].reshape(PT, 1).astype(np.float32)))
    W1T = np.asarray(W1, np.float64).T        # [144, 100]
    consts["w1_m"] = bf(W1T[:PM])
    consts["w1_t4"] = bf(_strip4(W1T[PM:].astype(np.float64)))  # [128, 100]
    consts["w2"] = bf(np.asarray(W2, np.float64).T)   # [100, 100]
    consts["w3"] = bf(np.asarray(W3, np.float64).T)   # [100, 1]
    consts["b2"] = f32(np.asarray(b2).reshape(NSQ, 1))
    consts["b3"] = f32(np.asarray(b3).reshape(1, 1))
    return consts


CONST_SPECS = (
    [(f"{n}_{bk}", shp, BF16)
     for n in ("csum", "cne", "cemp", "cne2")
     for bk, shp in (("mm", [PM, PM]), ("tm4", [PM, PM]),
                     ("mt", [PM, PT]), ("tt4", [PM, PT]))]
    + [("t_m", [PM, 1], F32), ("t_t4", [PM, 1], F32),
       ("w1_m", [PM, NSQ], BF16), ("w1_t4", [PM, NSQ], BF16),
       ("w2", [NSQ, NSQ], BF16), ("w3", [NSQ, 1], BF16),
       ("b2", [NSQ, 1], F32), ("b3", [1, 1], F32)]
)


# ---------------------------------------------------------------- device kernel

def emit_kernel(nc):
    xm_d = nc.dram_tensor("xm", [PM, BC], BF16, kind="ExternalInput")
    xt_d = nc.dram_tensor("xt4", [PM, DB], BF16, kind="ExternalInput")
    out_d = nc.dram_tensor("out", [1, BC], F32, kind="ExternalOutput")
    const_d = {n: nc.dram_tensor(n, shp, dt, kind="ExternalInput")
               for n, shp, dt in CONST_SPECS}

    with tile.TileContext(nc) as tc, ExitStack() as ctx:
        cpool = ctx.enter_context(tc.tile_pool(name="consts", bufs=1))
        xpool = ctx.enter_context(tc.tile_pool(name="x", bufs=1))
        qpool = ctx.enter_context(tc.tile_pool(name="perq", bufs=1))
        shpool = ctx.enter_context(tc.tile_pool(name="shared", bufs=2))
        npool = ctx.enter_context(tc.tile_pool(name="nscr", bufs=3))
        gpool = ctx.enter_context(tc.tile_pool(name="g", bufs=1))
        opool = ctx.enter_context(tc.tile_pool(name="outs", bufs=1))
        ppool = ctx.enter_context(tc.tile_pool(name="ps", bufs=1, space="PSUM"))
        tpool = ctx.enter_context(tc.tile_pool(name="pst", bufs=1, space="PSUM"))

        # ACT table warmup with no data deps
        warm = cpool.tile([1, 1], F32, tag="warm", name="warm")
        nc.vector.memset(warm[:], 0.0)
        nc.scalar.activation(warm[:], warm[:], AF.Copy)
        nc.scalar.activation(warm[:], warm[:], AF.Sigmoid)
        nc.scalar.activation(warm[:], warm[:], AF.Tanh)

        C = {}
        for n, shp, dt in CONST_SPECS:
            t = cpool.tile(shp, dt, tag=n, name=n)
            nc.gpsimd.dma_start(t[:], const_d[n][:])
            C[n] = t

        xm = xpool.tile([PM, BC], BF16, tag="xm", name="xm_t")
        xt = xpool.tile([PM, DB], BF16, tag="xt", name="xt_t")
        nc.gpsimd.dma_start(xm[:], xm_d[:])
        nc.gpsimd.dma_start(xt[:], xt_d[:])

        NQ = NDB + 1   # 4 main dblocks + 1 tail set (index NDB)

        def xq(q):
            # x source for set q: main slice or tail pack
            return xm[:, q * DB:(q + 1) * DB] if q < NDB else xt[:]

        # ---- persistent per-set tiles -------------------------------------
        mstk = [qpool.tile([PM, NK * DB], BF16, tag=f"mstk{q}", name=f"mstk{q}")
                for q in range(NQ)]          # masks m_1..m_5 stacked on free dim
        necn = [qpool.tile([PM, NK * DB], BF16, tag=f"necn{q}", name=f"necn{q}")
                for q in range(NQ)]
        s_t = [qpool.tile([PM, DB], BF16, tag=f"s{q}", name=f"s{q}")
               for q in range(NQ)]
        allv = [qpool.tile([PM, DB], BF16, tag=f"allv{q}", name=f"allv{q}")
                for q in range(NQ)]
        ebar = [qpool.tile([PM, DB], BF16, tag=f"ebar{q}", name=f"ebar{q}")
                for q in range(NQ)]
        sel = [qpool.tile([PM, DB], BF16, tag=f"sel{q}", name=f"sel{q}")
               for q in range(NQ)]
        t0_t = [qpool.tile([PM, DB], BF16, tag=f"t0{q}", name=f"t0{q}")
                for q in range(NQ)]

        def msl(q, c):
            return mstk[q][:, (c - 1) * DB:c * DB]

        def mint(q, c):
            return msl(q, c).bitcast(I16)

        # ---- conv stage helper --------------------------------------------
        def conv_stage(mat, rhs_main, rhs_tail, evac_main, evac_tail):
            """rhs_main(j)->AP [128, DB]; rhs_tail->AP [128, DB] pack (strips).
            evac_main(j, ps), evac_tail(ps)."""
            for j in range(NDB):
                ps = ppool.tile([PM, DB], F32, tag=f"pm{j % 3}", name=f"pm{j % 3}")
                rm = rhs_main(j)
                for nn in (0, 512):
                    nc.tensor.matmul(ps[:, nn:nn + 512], C[f"{mat}_mm"][:],
                                     rm[:, nn:nn + 512], start=True, stop=False)
                for nn in (0, 512):
                    nc.tensor.matmul(ps[:, nn:nn + 512],
                                     C[f"{mat}_tm4"][32 * j:32 * j + PT, :],
                                     rhs_tail[32 * j:32 * j + PT, nn:nn + 512],
                                     start=False, stop=True,
                                     tile_position=(32 * j, 0),
                                     skip_group_check=True)
                evac_main(j, ps)
            pst = tpool.tile([PM, DB], F32, tag="pt", name="pt")
            for nn in (0, 512):
                for j in range(NDB):
                    rm = rhs_main(j)
                    nc.tensor.matmul(pst[32 * j:32 * j + PT, nn:nn + 512],
                                     C[f"{mat}_mt"][:],
                                     rm[:, nn:nn + 512], start=True, stop=False,
                                     tile_position=(0, 32 * j),
                                     skip_group_check=True)
                for j in range(NDB):
                    nc.tensor.matmul(pst[32 * j:32 * j + PT, nn:nn + 512],
                                     C[f"{mat}_tt4"][32 * j:32 * j + PT, :],
                                     rhs_tail[32 * j:32 * j + PT, nn:nn + 512],
                                     start=False, stop=True,
                                     tile_position=(32 * j, 32 * j),
                                     skip_group_check=True)
            evac_tail(pst)

        # ---- phase A: masks ----------------------------------------------
        e_t = [qpool.tile([PM, DB], BF16, tag=f"e{q}", name=f"e{q}")
               for q in range(NQ)]
        for q in range(NQ):
            x = xq(q)
            nc.vector.tensor_scalar(e_t[q][:], x, 0, None, ALU.is_equal)
            for c in range(1, NK + 1):
                nc.vector.tensor_scalar(msl(q, c), x, c, None, ALU.is_equal)

        # ---- phase B: e-convs --------------------------------------------
        def ev_t0_m(j, ps):
            nc.scalar.activation(t0_t[j][:], ps[:], AF.Identity,
                                 bias=C["t_m"][:], scale=-1.0)

        def ev_t0_t(ps):
            nc.scalar.activation(t0_t[NDB][:], ps[:], AF.Identity,
                                 bias=C["t_t4"][:], scale=-1.0)

        conv_stage("cne", lambda j: e_t[j][:], e_t[NDB][:], ev_t0_m, ev_t0_t)

        def ev_e0(q, ps):
            e0 = shpool.tile([PM, DB], BF16, tag="e0", name=f"e0{q}")
            ie = shpool.tile([PM, DB], BF16, tag="ie", name=f"ie{q}")
            nc.scalar.activation(e0[:], ps[:], AF.Copy)
            nc.vector.tensor_scalar(ie[:], xq(q), 0, None, ALU.not_equal)
            # ebar = ie*E0 ; ew = E0-t0 (in place) ; s = ie*ew
            nc.vector.tensor_tensor(ebar[q][:], ie[:], e0[:], ALU.mult)
            nc.vector.tensor_tensor(e0[:], e0[:], t0_t[q][:], ALU.subtract)
            nc.vector.tensor_tensor(s_t[q][:], ie[:], e0[:], ALU.mult)

        conv_stage("cemp", lambda j: e_t[j][:], e_t[NDB][:],
                   lambda j, ps: ev_e0(j, ps), lambda ps: ev_e0(NDB, ps))

        # ---- phase C: per-color convs ------------------------------------
        for c in range(1, NK + 1):
            def ev_n(q, ps, c=c):
                nscr = npool.tile([PM, DB], BF16, tag="n", name=f"n{q}")
                nc.scalar.activation(nscr[:], ps[:], AF.Copy)
                nc.vector.tensor_tensor(necn[q][:, (c - 1) * DB:c * DB],
                                        nscr[:], t0_t[q][:], ALU.subtract)

            conv_stage("cne", lambda j, c=c: msl(j, c), msl(NDB, c),
                       lambda j, ps: ev_n(j, ps), lambda ps: ev_n(NDB, ps))

            def ev_u(q, ps, c=c):
                if c == 1:
                    nc.vector.tensor_tensor(sel[q][:], msl(q, c), ps[:], ALU.mult)
                else:
                    nc.vector.copy_predicated(sel[q][:], mint(q, c), ps[:])

            conv_stage("csum", lambda j, c=c: msl(j, c), msl(NDB, c),
                       lambda j, ps: ev_u(j, ps), lambda ps: ev_u(NDB, ps))

        # ---- s0 / sigmoid -------------------------------------------------
        for q in range(NQ):
            nc.vector.tensor_tensor(s_t[q][:], s_t[q][:], sel[q][:], ALU.add)
            nc.scalar.activation(allv[q][:], s_t[q][:], AF.Sigmoid)

        # ---- depth loop x2 ------------------------------------------------
        for it in range(2):
            for c in range(1, NK + 1):
                gt = [gpool.tile([PM, DB], BF16, tag=f"g{q}", name=f"g{q}")
                      for q in range(NQ)]
                for q in range(NQ):
                    nc.vector.tensor_tensor(gt[q][:],
                                            necn[q][:, (c - 1) * DB:c * DB],
                                            allv[q][:], ALU.mult)

                def ev_a(q, ps, c=c):
                    if c == 1:
                        nc.vector.tensor_tensor(sel[q][:], msl(q, c), ps[:],
                                                ALU.mult)
                    else:
                        nc.vector.copy_predicated(sel[q][:], mint(q, c), ps[:])

                conv_stage("cne2", lambda j: gt[j][:], gt[NDB][:],
                           lambda j, ps: ev_a(j, ps), lambda ps: ev_a(NDB, ps))
            for q in range(NQ):
                nc.vector.tensor_tensor(s_t[q][:], s_t[q][:], ebar[q][:], ALU.add)
                nc.vector.tensor_tensor(s_t[q][:], s_t[q][:], sel[q][:], ALU.subtract)
                nc.scalar.activation(allv[q][:], s_t[q][:], AF.Tanh, scale=0.5)

        # ---- MLP ----------------------------------------------------------
        for j in range(NDB):
            ps1 = ppool.tile([PM, DB], F32, tag=f"pm{j % 3}", name=f"mlp1_{j}")
            for nn in (0, 512):
                nc.tensor.matmul(ps1[:NSQ, nn:nn + 512], C["w1_m"][:],
                                 allv[j][:, nn:nn + 512], start=True, stop=False)
                nc.tensor.matmul(ps1[:NSQ, nn:nn + 512],
                                 C["w1_t4"][32 * j:32 * j + PT, :],
                                 allv[NDB][32 * j:32 * j + PT, nn:nn + 512],
                                 start=False, stop=True,
                                 tile_position=(32 * j, 0),
                                 skip_group_check=True)
            h1 = opool.tile([NSQ, DB], BF16, tag="h1", name="h1")
            nc.scalar.activation(h1[:], ps1[:NSQ], AF.Copy)
            nc.vector.scalar_tensor_tensor(h1[:], ps1[:NSQ], 0.2, h1[:],
                                           ALU.mult, ALU.max)
            ps2 = tpool.tile([PM, DB], F32, tag="pt", name=f"mlp2_{j}")
            for nn in (0, 512):
                nc.tensor.matmul(ps2[:NSQ, nn:nn + 512], C["w2"][:],
                                 h1[:, nn:nn + 512], start=True, stop=True)
            h2 = opool.tile([NSQ, DB], BF16, tag="h2", name="h2")
            nc.scalar.activation(h2[:], ps2[:NSQ], AF.Identity, bias=C["b2"][:])
            nc.vector.scalar_tensor_tensor(h2[:], h2[:], 0.2, h2[:],
                                           ALU.mult, ALU.max)
            ps3 = ppool.tile([PM, DB], F32, tag=f"pm{(j + 1) % 3}", name=f"mlp3_{j}")
            for nn in (0, 512):
                nc.tensor.matmul(ps3[:1, nn:nn + 512], C["w3"][:],
                                 h2[:, nn:nn + 512], start=True, stop=True)
            yout = opool.tile([1, DB], F32, tag="yout", name="yout")
            nc.scalar.activation(yout[:], ps3[:1], AF.Identity, bias=C["b3"][:])
            nc.gpsimd.dma_start(out_d[:, j * DB:(j + 1) * DB], yout[:])

    return nc


# ---------------------------------------------------------------- entry point

def _prep_inputs(dots):
    """dots (14,12,B) int32 -> per-core xm [128, BC] bf16, xt4 [128, DB] bf16."""
    x = np.asarray(dots)[:12].reshape(NPIX, B).astype(ml_dtypes.bfloat16)
    xms, xts = [], []
    for k in range(NCORES):
        sl = x[:, k * BC:(k + 1) * BC]
        xms.append(np.ascontiguousarray(sl[:PM]))
        tail = sl[PM:]                       # [16, 4096]
        pack = np.zeros((PM, DB), ml_dtypes.bfloat16)
        for j in range(NDB):
            pack[32 * j:32 * j + PT] = tail[:, j * DB:(j + 1) * DB]
        xts.append(pack)
    return xms, xts


def kernel(dots, w_each, w_not_each, w_not_each_2nd, w_empty, W1, W2, b2, W3, b3):
    from concourse.bass_utils import run_bass_kernel_spmd

    consts = build_consts(w_each, w_not_each, w_not_each_2nd, w_empty,
                          W1, W2, b2, W3, b3)
    xms, xts = _prep_inputs(dots)

    nc = bacc.Bacc()
    emit_kernel(nc)
    nc.compile()

    in_maps = [dict(consts, xm=xms[k], xt4=xts[k]) for k in range(NCORES)]
    res = run_bass_kernel_spmd(nc, in_maps, list(range(NCORES)))
    out = np.concatenate([np.asarray(r["out"]).reshape(BC) for r in res.results])
    return out.reshape(B, 1).astype(np.float32)


if __name__ == "__main__":
    rng = np.random.default_rng(0)
    ins = {
        "dots": rng.integers(0, 6, size=(V, H, B)).astype(np.int32),
        "w_each": rng.standard_normal((FULL, CT), dtype=np.float32) * 0.1,
        "w_not_each": rng.standard_normal((FULL, CT), dtype=np.float32) * 0.1,
        "w_not_each_2nd": rng.standard_normal((FULL, CT), dtype=np.float32) * 0.1,
        "w_empty": rng.standard_normal((FULL, CT), dtype=np.float32) * 0.1,
        "W1": rng.standard_normal((NSQ, NPIX), dtype=np.float32) * 0.2,
        "W2": rng.standard_normal((NSQ, NSQ), dtype=np.float32) * 0.2,
        "b2": rng.standard_normal(NSQ, dtype=np.float32) * 0.1,
        "W3": rng.standard_normal((1, NSQ), dtype=np.float32) * 0.2,
        "b3": rng.standard_normal(1, dtype=np.float32) * 0.1,
    }
    y = kernel(**ins)
    print("kernel out", y.shape, y[:4, 0])
